# revision 38
# baseline (speedup 1.0000x reference)
"""TRN2 Bass kernel for nn_Attention_52012053955159.

Reference math:
    Q = x @ W_q[h]; K = x @ W_k[h]; V = x @ W_v[h]       (per head h)
    scores = Q K^T with scores[i,j] = -1e9 where mask[i] | mask[j]
    values = scores @ V          (no softmax)
    out = sum_h values_h @ W_o[h]

Algorithm. With keep = ~mask (n_keep ~ S/2):
  * masked OUTPUT rows i:   out[i] = sum_h (-1e9 * Vsum_all_h) @ W_o[h]
      -- one shared row vector ("mrow").
  * unmasked OUTPUT rows i: out[i] = sum_h [ (Q K^T restricted to unmasked
      rows/cols) @ V  + (-1e9) * Vsum_masked_h ] @ W_o[h]
So: compact x to its unmasked rows (host gather, zero-padded to S1=1152),
run dense attention on the compacted sequence, add the per-head
-1e9*Vsum_masked correction (constant along the row axis -> one
tensor_scalar_add), and compute mrow on device with two tiny extra matmul
rows. Host scatters rows back. The device program is mask-independent.

Sharding: 8 cores = 4 batches x 2 head-halves (8 heads each); host adds the
two half partials per batch.

Precision: output is dominated by the -1e9 * Vsum terms, so the V
projection and W_o contraction use hi/lo bf16 3-term decomposition
(x ~ xh + xl; x@w ~ xh@wh + xh@wl + xl@wh, ~1.5e-5 rel err). Vsum_all is
computed from fp32 column sums of x (split hi/lo for the matmul).
Q/K/scores run in plain bf16: their error only enters the compacted QK@V
term, which is ~1e-7 of output scale.
"""
import numpy as np
import ml_dtypes

import concourse.bass as bass
import concourse.mybir as mybir
import concourse.tile as tile
from concourse.bass_utils import run_bass_kernel_spmd

f32 = mybir.dt.float32
bf = mybir.dt.bfloat16
bf16 = ml_dtypes.bfloat16

B, S, DIN, H, DK, DV, DOUT = 4, 2048, 1024, 16, 64, 64, 1024
NCORES = 8
HPC = 8          # heads per core
NPAIR = 4        # head pairs per core
NDC = DIN // 128     # 8 contraction chunks
S1 = 1152            # compacted+padded sequence length (9 x 128)
NST1 = S1 // 128     # 9
IT_TILES = [(0, 512), (512, 512), (1024, 128)]   # free-dim tiles of S1
NEG = -1e9

MULT = mybir.AluOpType.mult
ADD = mybir.AluOpType.add
SUB = mybir.AluOpType.subtract
AX_X = mybir.AxisListType.X


# ---------------------------------------------------------------------------
# Wait legalization: this walrus build accepts at most ONE sync wait per
# instruction; split extras onto preceding same-engine NoOps.
def _legalize_waits(nc):
    ctr = 0
    for f in nc.m.functions:
        for bb in f.blocks:
            new_insts = []
            changed = False
            for inst in bb.instructions:
                si = getattr(inst, "sync_info", None)
                waits = list(si.on_wait) if si is not None and si.on_wait else []
                if len(waits) > 1:
                    for w in waits[:-1]:
                        ctr += 1
                        nop = mybir.InstNoOp(name=f"legal-nop-{ctr}", ins=[], outs=[])
                        nop.engine = inst.engine
                        nop.sync_info = mybir.SyncInfo(on_wait=[w], on_update=[])
                        new_insts.append(nop)
                    inst.sync_info = mybir.SyncInfo(
                        on_wait=[waits[-1]], on_update=list(si.on_update)
                    )
                    changed = True
                new_insts.append(inst)
            if changed:
                bb.instructions[:] = new_insts
    return ctr


# ---------------------------------------------------------------------------
def _build_bass():
    nc = bass.Bass("TRN2", target_bir_lowering=False, debug=False)

    xh = nc.dram_tensor("xh", [DIN, S1], bf, kind="ExternalInput").ap()
    xl = nc.dram_tensor("xl", [DIN, S1], bf, kind="ExternalInput").ap()
    xmh = nc.dram_tensor("xmh", [DIN, S1], bf, kind="ExternalInput").ap()
    xml = nc.dram_tensor("xml", [DIN, S1], bf, kind="ExternalInput").ap()
    wqs = nc.dram_tensor("wqs", [DIN, HPC * DK], bf, kind="ExternalInput").ap()
    wks = nc.dram_tensor("wks", [DIN, HPC * DK], bf, kind="ExternalInput").ap()
    wvh = nc.dram_tensor("wvh", [DIN, HPC * DV], bf, kind="ExternalInput").ap()
    wvl = nc.dram_tensor("wvl", [DIN, HPC * DV], bf, kind="ExternalInput").ap()
    woh = nc.dram_tensor("woh", [HPC * DV, DOUT], bf, kind="ExternalInput").ap()
    wol = nc.dram_tensor("wol", [HPC * DV, DOUT], bf, kind="ExternalInput").ap()
    outc = nc.dram_tensor("outc", [S1, DOUT], f32, kind="ExternalOutput").ap()
    extra = nc.dram_tensor("extra", [2, DOUT], f32, kind="ExternalOutput").ap()

    xh_r = xh.rearrange("(dc p) s -> p dc s", p=128)
    xl_r = xl.rearrange("(dc p) s -> p dc s", p=128)
    xmh_r = xmh.rearrange("(dc p) s -> p dc s", p=128)
    xml_r = xml.rearrange("(dc p) s -> p dc s", p=128)
    wqs_r = wqs.rearrange("(dc p) c -> p dc c", p=128)
    wks_r = wks.rearrange("(dc p) c -> p dc c", p=128)
    wvh_r = wvh.rearrange("(dc p) c -> p dc c", p=128)
    wvl_r = wvl.rearrange("(dc p) c -> p dc c", p=128)
    woh_r = woh.rearrange("(pr r) o -> r pr o", r=128)
    wol_r = wol.rearrange("(pr r) o -> r pr o", r=128)

    with tile.TileContext(nc) as tc:
        with (
            tc.tile_pool(name="big", bufs=1) as big,
            tc.tile_pool(name="wop", bufs=1) as wop,
            tc.tile_pool(name="outp", bufs=2) as outp,
            tc.tile_pool(name="smalls", bufs=1) as smalls,
        ):
            # DMA order tuned for the consumption schedule: V weights
            # (per-dc chunks) + first x slice first, bulk x next, per-pair
            # Q/K weights, masked-x, and W_o last (needed only ~170us in).
            # Latency-critical loads on the Pool queue (cheap issue) in
            # exact need-order; bulk/late loads on the SP queue.
            wvh_sb = big.tile([128, NDC, HPC * DV], bf, tag="wvh")
            wvl_sb = big.tile([128, NDC, HPC * DV], bf, tag="wvl")
            xh_sb = big.tile([128, NDC, S1], bf, tag="xh")
            xl_sb = big.tile([128, NDC, S1], bf, tag="xl")
            nc.sync.dma_start(wvh_sb, wvh_r)
            nc.sync.dma_start(xh_sb[:, :, 0:128], xh_r[:, :, 0:128])
            nc.sync.dma_start(wvl_sb, wvl_r)
            nc.sync.dma_start(xl_sb[:, :, 0:128], xl_r[:, :, 0:128])
            for c0 in range(128, S1, 256):
                c1 = min(c0 + 256, S1)
                nc.gpsimd.dma_start(xh_sb[:, :, c0:c1], xh_r[:, :, c0:c1])
                nc.gpsimd.dma_start(xl_sb[:, :, c0:c1], xl_r[:, :, c0:c1])
            wq_sbs, wk_sbs = [], []
            for pr in range(NPAIR):
                csl = slice(pr * 128, (pr + 1) * 128)
                wq_sb = big.tile([128, NDC, 128], bf, tag=f"wq{pr}",
                                 name=f"wq{pr}")
                nc.gpsimd.dma_start(wq_sb, wqs_r[:, :, csl])
                wk_sb = big.tile([128, NDC, 128], bf, tag=f"wk{pr}",
                                 name=f"wk{pr}")
                nc.gpsimd.dma_start(wk_sb, wks_r[:, :, csl])
                wq_sbs.append(wq_sb)
                wk_sbs.append(wk_sb)


            vbf_sb = big.tile([128, NST1, HPC * DV], bf, tag="vbf")
            vth = big.tile([128, NPAIR, S1], bf, tag="vth")
            vtl = big.tile([128, NPAIR, S1], bf, tag="vtl")

            xsum_sb = smalls.tile([128, NDC], f32, tag="xsum")
            xsh_sb = smalls.tile([128, NDC], bf, tag="xsh")
            xsl_sb = smalls.tile([128, NDC], bf, tag="xsl")
            AB_sb = smalls.tile([128, NPAIR], f32, tag="AB")
            mvhl_sb = smalls.tile([128, NPAIR, 2], bf, tag="mvhl")
            s_all = smalls.tile([128, NPAIR], f32, tag="s_all")
            s_unm = smalls.tile([128, NPAIR], f32, tag="s_unm")
            sc1 = smalls.tile([128, NDC], f32, tag="sc1")
            sc2 = smalls.tile([128, NPAIR], f32, tag="sc2")

            # ---------------- stage 1: V projection (hi/lo) + sums -------
            with (
                tc.tile_pool(name="vfp", bufs=3) as vfp,
                tc.tile_pool(name="ps1", bufs=2, space="PSUM") as ps1,
                tc.tile_pool(name="psv", bufs=1, space="PSUM") as psv,
            ):
                ones_sb = smalls.tile([128, 1], f32, tag="ones")
                nc.vector.memset(ones_sb, 1.0)
                vs_ps = [
                    psv.tile([128, 2], f32, tag=f"vs{p}", name=f"vs{p}")
                    for p in range(NPAIR)
                ]
                vsum_ps = [t[:, 0:1] for t in vs_ps]
                vall_ps = [t[:, 1:2] for t in vs_ps]
                for st in range(NST1):
                    ssl = slice(st * 128, (st + 1) * 128)
                    vps = ps1.tile([128, HPC * DV], f32, tag="vps")
                    # term-outer so the first 8 matmuls need only wvh + x slice
                    terms = [
                        (xh_sb, wvh_sb), (xl_sb, wvh_sb), (xh_sb, wvl_sb),
                    ]
                    for t, (lhs, rhs) in enumerate(terms):
                        for dc in range(NDC):
                            nc.tensor.matmul(
                                vps, lhs[:, dc, ssl], rhs[:, dc],
                                start=(dc == 0 and t == 0),
                                stop=(dc == NDC - 1 and t == 2),
                            )
                    vf = vfp.tile([128, HPC * DV], f32, tag="vf")
                    nc.vector.tensor_copy(vf, vps)
                    nc.scalar.copy(vbf_sb[:, st], vps)
                    for p in range(NPAIR):
                        # Vsum_unm[hv] += sum_j V[j, hv]  (fp32 matmul)
                        nc.tensor.matmul(
                            vsum_ps[p],
                            vf[:, p * 128:(p + 1) * 128],
                            ones_sb,
                            start=(st == 0),
                            stop=(st == NST1 - 1),
                        )

                # column sums of the MASKED rows (fp32), streamed in 3
                # chunks: xmsum[d] = sum_s xmh + sum_s xml
                XMC = S1 // 3
                with tc.tile_pool(name="xmp", bufs=2) as xmp:
                    first = True
                    for c in range(3):
                        cs = slice(c * XMC, (c + 1) * XMC)
                        for src_r in (xmh_r, xml_r):
                            xm_t = xmp.tile([128, NDC, XMC], bf, tag="xm")
                            nc.gpsimd.dma_start(xm_t, src_r[:, :, cs])
                            for dc in range(NDC):
                                nc.vector.reduce_sum(
                                    sc1[:, dc:dc + 1], xm_t[:, dc], AX_X
                                )
                            if first:
                                nc.vector.tensor_copy(xsum_sb, sc1)
                                first = False
                            else:
                                nc.vector.tensor_add(xsum_sb, xsum_sb, sc1)
                nc.vector.tensor_copy(xsh_sb, xsum_sb)
                nc.scalar.copy(sc1, xsh_sb)      # bf16 -> f32 on ACT
                nc.vector.tensor_tensor(xsl_sb, xsum_sb, sc1, SUB)

                # Vsum_masked[hv] = sum_dc wv[dc].T @ xmsum[dc] (hi/lo)
                for p in range(NPAIR):
                    csl = slice(p * 128, (p + 1) * 128)
                    n = 0
                    for dc in range(NDC):
                        terms = [
                            (wvh_sb[:, dc, csl], xsh_sb[:, dc:dc + 1]),
                            (wvh_sb[:, dc, csl], xsl_sb[:, dc:dc + 1]),
                            (wvl_sb[:, dc, csl], xsh_sb[:, dc:dc + 1]),
                        ]
                        for lh, rh in terms:
                            n += 1
                            nc.tensor.matmul(
                                vall_ps[p], lh, rh,
                                start=(n == 1), stop=(n == 3 * NDC),
                            )
                # corrections: AB = -1e9*Vsum_masked;
                # mrow vec = -1e9*(Vsum_masked + Vsum_unm) (hi/lo split)
                for p in range(NPAIR):
                    nc.vector.tensor_copy(s_all[:, p:p + 1], vall_ps[p])
                    nc.vector.tensor_copy(s_unm[:, p:p + 1], vsum_ps[p])
                nc.vector.tensor_scalar_mul(AB_sb, s_all, NEG)
                nc.vector.tensor_tensor(sc2, s_all, s_unm, ADD)
                nc.vector.tensor_scalar_mul(sc2, sc2, NEG)
                nc.vector.tensor_copy(mvhl_sb[:, :, 0], sc2)
                nc.scalar.copy(s_all, mvhl_sb[:, :, 0])  # bf16 -> f32
                nc.vector.tensor_tensor(s_unm, sc2, s_all, SUB)
                nc.vector.tensor_copy(mvhl_sb[:, :, 1], s_unm)

            # W_o weights (needed from stage 3 onwards; DMA has slack here)
            woh_sb = wop.tile([128, NPAIR, DOUT], bf, tag="woh")
            nc.gpsimd.dma_start(woh_sb, woh_r)
            wol_sb = wop.tile([128, NPAIR, DOUT], bf, tag="wol")
            nc.gpsimd.dma_start(wol_sb, wol_r)

            # ---------------- stage 2: per head-pair QK + scores + values
            with (
                tc.tile_pool(name="qkp", bufs=2) as qkp,
                tc.tile_pool(name="sTp", bufs=4) as sTp,
                tc.tile_pool(name="psqk", bufs=2, space="PSUM") as psqk,
                tc.tile_pool(name="pss", bufs=2, space="PSUM") as pss,
                tc.tile_pool(name="psvt", bufs=2, space="PSUM") as psvt,
            ):
                for pr in range(NPAIR):
                    wq_sb = wq_sbs[pr]
                    wk_sb = wk_sbs[pr]
                    qT = qkp.tile([128, S1], bf, tag="qT")
                    kT = qkp.tile([128, S1], bf, tag="kT")
                    for off, w in IT_TILES:
                        isl = slice(off, off + w)
                        qps = psqk.tile([128, 512], f32, tag="qk")
                        for dc in range(NDC):
                            nc.tensor.matmul(
                                qps[:, :w], wq_sb[:, dc], xh_sb[:, dc, isl],
                                start=(dc == 0), stop=(dc == NDC - 1),
                            )
                        nc.vector.tensor_copy(qT[:, isl], qps[:, :w])
                        kps = psqk.tile([128, 512], f32, tag="qk")
                        for dc in range(NDC):
                            nc.tensor.matmul(
                                kps[:, :w], wk_sb[:, dc], xh_sb[:, dc, isl],
                                start=(dc == 0), stop=(dc == NDC - 1),
                            )
                        nc.scalar.copy(kT[:, isl], kps[:, :w])

                    for off, w in IT_TILES:
                        isl = slice(off, off + w)
                        vtps = psvt.tile([128, 512], f32, tag="vt")
                        for jt in range(NST1):
                            jsl = slice(jt * 128, (jt + 1) * 128)
                            psA = pss.tile([128, 512], f32, tag="psA")
                            psB = pss.tile([128, 512], f32, tag="psB")
                            nc.tensor.matmul(
                                psA[:, :w], kT[0:64, jsl], qT[0:64, isl],
                                start=True, stop=True, tile_position=(0, 0),
                            )
                            nc.tensor.matmul(
                                psB[:, :w], kT[64:128, jsl], qT[64:128, isl],
                                start=True, stop=True, tile_position=(64, 0),
                            )
                            sA = sTp.tile([128, 512], bf, tag="sA")
                            sB = sTp.tile([128, 512], bf, tag="sB")
                            nc.vector.tensor_copy(sA[:, :w], psA[:, :w])
                            nc.scalar.copy(sB[:, :w], psB[:, :w])
                            nc.tensor.matmul(
                                vtps[0:64, :w],
                                vbf_sb[:, jt, pr * 128:pr * 128 + 64],
                                sA[:, :w],
                                start=(jt == 0), stop=(jt == NST1 - 1),
                                tile_position=(0, 0),
                            )
                            nc.tensor.matmul(
                                vtps[64:128, :w],
                                vbf_sb[:, jt, pr * 128 + 64:pr * 128 + 128],
                                sB[:, :w],
                                start=(jt == 0), stop=(jt == NST1 - 1),
                                tile_position=(0, 64),
                            )
                        # add -1e9*Vsum_masked (constant along i) and split
                        # hi/lo for the W_o stage
                        vt_t = outp.tile([128, 512], f32, tag="vtt")
                        nc.vector.tensor_scalar_add(
                            vt_t[:, :w], vtps[:, :w], AB_sb[:, pr:pr + 1],
                        )
                        nc.vector.tensor_copy(vth[:, pr, isl], vt_t[:, :w])
                        t = outp.tile([128, 512], f32, tag="tmp")
                        nc.scalar.copy(t[:, :w], vth[:, pr, isl])
                        nc.vector.tensor_tensor(
                            vtl[:, pr, isl], vt_t[:, :w], t[:, :w], SUB
                        )

            # ---------------- stage 3: W_o contraction (hi/lo) -----------
            with tc.tile_pool(name="pso", bufs=2, space="PSUM") as pso:
                # masked-row vector first (inputs ready early): rows
                # (mh, ml) @ (woh + wol)
                psx = pso.tile([2, DOUT], f32, tag="psx")
                for dt_ in range(2):
                    osl = slice(dt_ * 512, (dt_ + 1) * 512)
                    nn_ = 0
                    for pr in range(NPAIR):
                        for rh in (woh_sb[:, pr, osl], wol_sb[:, pr, osl]):
                            nn_ += 1
                            nc.tensor.matmul(
                                psx[:, osl], mvhl_sb[:, pr], rh,
                                start=(nn_ == 1), stop=(nn_ == 2 * NPAIR),
                            )
                xb = outp.tile([2, DOUT], f32, tag="xb")
                nc.vector.tensor_copy(xb, psx)
                nc.sync.dma_start(extra, xb)

                for st in range(NST1):
                    ssl = slice(st * 128, (st + 1) * 128)
                    ops = pso.tile([128, DOUT], f32, tag="ops")
                    for dt_ in range(2):
                        osl = slice(dt_ * 512, (dt_ + 1) * 512)
                        for pr in range(NPAIR):
                            terms = [
                                (vth[:, pr, ssl], woh_sb[:, pr, osl]),
                                (vth[:, pr, ssl], wol_sb[:, pr, osl]),
                                (vtl[:, pr, ssl], woh_sb[:, pr, osl]),
                            ]
                            for t, (lh, rh) in enumerate(terms):
                                nc.tensor.matmul(
                                    ops[:, osl], lh, rh,
                                    start=(pr == 0 and t == 0),
                                    stop=(pr == NPAIR - 1 and t == 2),
                                )
                    ob = outp.tile([128, DOUT], f32, tag="ob")
                    if st % 2 == 0:
                        nc.vector.tensor_copy(ob, ops)
                        nc.sync.dma_start(outc[ssl, :], ob)
                    else:
                        nc.scalar.copy(ob, ops)
                        nc.gpsimd.dma_start(outc[ssl, :], ob)

    _legalize_waits(nc)
    return nc


_NC_CACHE = None


def _get_nc():
    global _NC_CACHE
    if _NC_CACHE is None:
        _NC_CACHE = _build_bass()
    return _NC_CACHE


def _hi_lo(a):
    h = a.astype(bf16)
    l = (a - h.astype(np.float32)).astype(bf16)
    return h, l


def kernel_in_maps(x, mask, W_q, W_k, W_v, W_o):
    x = np.asarray(x, dtype=np.float32)
    mask_b = np.asarray(mask).astype(bool)
    W_q = np.asarray(W_q, dtype=np.float32)
    W_k = np.asarray(W_k, dtype=np.float32)
    W_v = np.asarray(W_v, dtype=np.float32)
    W_o = np.asarray(W_o, dtype=np.float32)

    keep_idx = [np.flatnonzero(~mask_b[b]) for b in range(B)]
    masked_idx = [np.flatnonzero(mask_b[b]) for b in range(B)]
    for b in range(B):
        assert len(keep_idx[b]) <= S1, (
            f"batch {b}: {len(keep_idx[b])} unmasked rows > S1={S1}"
        )
        assert len(masked_idx[b]) <= S1, (
            f"batch {b}: {len(masked_idx[b])} masked rows > S1={S1}"
        )

    in_maps = []
    for core in range(NCORES):
        b, hh = divmod(core, 2)
        heads = slice(hh * HPC, (hh + 1) * HPC)
        ki = keep_idx[b]
        xc = np.zeros((S1, DIN), np.float32)
        xc[:len(ki)] = x[b][ki]
        xcT = np.ascontiguousarray(xc.T)                     # [DIN, S1]
        xh, xl = _hi_lo(xcT)
        mi = masked_idx[b]
        xm = np.zeros((S1, DIN), np.float32)
        xm[:len(mi)] = x[b][mi]
        xmT = np.ascontiguousarray(xm.T)                     # [DIN, S1]
        xmh, xml = _hi_lo(xmT)
        wq_s = np.ascontiguousarray(
            W_q[heads].transpose(1, 0, 2).reshape(DIN, HPC * DK)
        ).astype(bf16)
        wk_s = np.ascontiguousarray(
            W_k[heads].transpose(1, 0, 2).reshape(DIN, HPC * DK)
        ).astype(bf16)
        wv_s = np.ascontiguousarray(
            W_v[heads].transpose(1, 0, 2).reshape(DIN, HPC * DV)
        )
        wvh, wvl = _hi_lo(wv_s)
        wo_s = np.ascontiguousarray(W_o[heads].reshape(HPC * DV, DOUT))
        woh, wol = _hi_lo(wo_s)
        in_maps.append({
            "xh": xh, "xl": xl, "xmh": xmh, "xml": xml,
            "wqs": wq_s, "wks": wk_s,
            "wvh": wvh, "wvl": wvl, "woh": woh, "wol": wol,
        })
    return in_maps, keep_idx


def kernel(x, mask, W_q, W_k, W_v, W_o, _trace=False, _trace_kwargs=None):
    in_maps, keep_idx = kernel_in_maps(x, mask, W_q, W_k, W_v, W_o)
    nc = _get_nc()
    kw = {}
    if _trace:
        kw["trace"] = True
        kw.update(_trace_kwargs or {})
    res = run_bass_kernel_spmd(nc, in_maps, core_ids=list(range(NCORES)), **kw)

    mask_b = np.asarray(mask).astype(bool)
    out = np.empty((B, S, DOUT), np.float32)
    for b in range(B):
        ra, rb = res.results[2 * b], res.results[2 * b + 1]
        ki = keep_idx[b]
        merged = ra["outc"][:len(ki)] + rb["outc"][:len(ki)]
        mrow = (ra["extra"][0] + ra["extra"][1]
                + rb["extra"][0] + rb["extra"][1])
        out[b] = mrow
        out[b][ki] = merged
    if _trace:
        kernel._last_results = res
    return out
